# revision 1
# baseline (speedup 1.0000x reference)
"""Trainium2 Bass kernel for CustomGATConv (dense masked GAT attention).

Strategy (8-core SPMD, row-sharded attention):
  - Each core owns 512 destination rows i of the [4096, 4096, 8] attention
    tensor.  Inputs are node-rotated per core so that the identical program
    always works on rows [0:512) of its own rotated node order.
  - h = x @ W is computed on every core (replicated, cheap on PE).
  - Per (row-block, head): z[j, i] = e_src[i] + e_dst[j] + (-200 if masked)
    is built entirely in PSUM by three tiny matmuls (rank-1/2 outer products
    plus an identity-weighted mask inject), so the ScalarEngine only runs
    two activation passes: Prelu(alpha=0.2) then Exp.  exp(-200ish) == 0
    implements the mask.
  - alpha @ h and the softmax denominator come from one accumulated matmul
    against h augmented with a ones column ([K=128 j, 65]).
  - Normalization: PE-transpose of the [65, 512] accumulator, then a DVE
    reciprocal + per-partition scalar multiply.
"""

import re

import numpy as np
import ml_dtypes

import bass_rust as br
import concourse.bass as bass
import concourse.tile as tile
from concourse import mybir
from concourse.bass_utils import run_bass_kernel_spmd

N = 4096
IN = 256
H = 8
F = 64
NCORES = 8
R = N // NCORES          # 512 destination rows per core
JT = N // 128            # 32 j-tiles
KC = IN // 128           # 2 contraction chunks for x @ W
NEG = -200.0             # additive mask value
FP = mybir.dt.float32
BF = mybir.dt.bfloat16
F16 = mybir.dt.float16


class _TileContext(tile.TileContext):
    """TileContext whose final drain splits its semaphore waits one per
    instruction — this walrus's CTRL_NO encoding only fits one sync wait."""

    def _drain_and_barrier(self, tick_clock, wait_clock):
        gc = tick_clock.global_clock
        vals = list(map(int, re.findall(r"\d+", repr(gc))))
        nonzero = [(i, t) for i, t in enumerate(vals) if t > 0]
        prev = br.VectorClock()
        partial = br.VectorClock()
        for i, t in nonzero:
            partial.require_at_least(i, t)
            inst = self.nc.sync.drain().ins
            wait_clock.add_sem_waits(
                inst,
                br.ScopedClock({None: partial.copy()}),
                br.ScopedClock({None: prev.copy()}),
            )
            prev = partial.copy()
        drain_inst = self.nc.sync.drain().ins
        wait_clock.add_sem_waits(
            drain_inst,
            br.ScopedClock({None: gc}),
            br.ScopedClock({None: prev.copy()}),
        )
        self.nc.all_engine_barrier()
        popped = self.nc._tile_sem_poison_stack.pop()
        assert popped is self._sem_poison
        self.nc.clear_and_free_semaphores(list(self.sems.allocated().values()))
        self.nc.all_engine_barrier()


def _split_excess_waits(nc, cap_compute=1, cap_nop=1):
    """This walrus encodes at most ~2 sync waits per compute instruction and
    1 per CTRL_NO (nop/drain).  Move excess waits onto injected same-engine
    nops placed immediately before the over-subscribed instruction."""
    n_split = 0
    for fn in nc.m.functions:
        for bb in fn.blocks:
            lst = bb.instructions
            i = 0
            while i < len(lst):
                inst = lst[i]
                si = inst.sync_info
                waits = list(si.on_wait) if si is not None else []
                is_ctrl = isinstance(inst, (mybir.InstNoOp, mybir.InstDrain))
                cap = cap_nop if is_ctrl else cap_compute
                if len(waits) > cap:
                    excess, keep = waits[:-cap], waits[-cap:]
                    for w in excess:
                        nop = mybir.InstNoOp(name=f"waitsplit-{nc.next_id()}")
                        nop.engine = inst.engine
                        nop.sync_info = br.SyncInfo(on_wait=[w], on_update=[])
                        lst.insert(i, nop)
                        i += 1
                        n_split += 1
                    inst.sync_info = br.SyncInfo(
                        on_wait=keep, on_update=list(si.on_update)
                    )
                i += 1
    return n_split


def _build_program(repeat=1):
    nc = bass.Bass("TRN2", target_bir_lowering=False, debug=False)
    ap = {}
    ap["xT"] = nc.dram_tensor("xT", [IN, N], FP, kind="ExternalInput").ap()
    ap["w"] = nc.dram_tensor("w", [IN, H * F], FP, kind="ExternalInput").ap()
    ap["wa"] = nc.dram_tensor("wa", [IN, 2 * H], FP, kind="ExternalInput").ap()
    ap["maskadd"] = nc.dram_tensor("maskadd", [N, R], BF, kind="ExternalInput").ap()
    ap["identb"] = nc.dram_tensor("identb", [128, 128], BF, kind="ExternalInput").ap()
    ap["identf"] = nc.dram_tensor("identf", [128, 128], FP, kind="ExternalInput").ap()
    ap["onesh"] = nc.dram_tensor("onesh", [1, H, R], FP, kind="ExternalInput").ap()
    out_ap = nc.dram_tensor("out", [R, H * F], FP, kind="ExternalOutput").ap()

    with _TileContext(nc) as tc:
        _emit(tc, nc, ap, out_ap, repeat)
    _split_excess_waits(nc)
    return nc


def _emit(tc, nc, ap, out_ap, repeat):
    from contextlib import ExitStack

    Act = mybir.ActivationFunctionType
    with ExitStack() as ctx:
        singles = ctx.enter_context(tc.tile_pool(name="singles", bufs=1))

        # ---- persistent tiles ----
        mask_sb = singles.tile([128, JT, R], BF)
        nc.sync.dma_start(mask_sb[:], ap["maskadd"].rearrange("(jt p) i -> p jt i", p=128))
        identb_sb = singles.tile([128, 128], BF)
        nc.sync.dma_start(identb_sb[:], ap["identb"])
        identf_sb = singles.tile([128, 128], FP)
        nc.sync.dma_start(identf_sb[:], ap["identf"])

        haug_sb = singles.tile([128, JT, H, F + 1], FP)
        nc.vector.memset(haug_sb[:, :, :, F:F + 1], 1.0)
        esd_sb = singles.tile([16, N], FP)
        # zsrc2[{0,32}, h, :] = e_src row of head h; zsrc2[{1,33}, h, :] =
        # ones — the K=2 z-matmul rhs, replicated at partition bases 0 and
        # 32 because lhsT and rhs must share their base partition.
        zsrc2 = singles.tile([34, H, R], FP)
        nc.sync.dma_start(out=zsrc2[1:2, :, :], in_=ap["onesh"])
        nc.sync.dma_start(out=zsrc2[33:34, :, :], in_=ap["onesh"])
        # dst_quad[:, s, :]: lhsT pairs (manual ping-pong on s).  Rows 0/32
        # are all-ones; rows 1/33 receive the two e_dst row slices by DMA
        # each iteration (PE lhsT base partition must be 0/32/64).
        dst_quad = singles.tile([34, 4, 128], FP)
        nc.vector.memset(dst_quad[0:1, :, :], 1.0)
        nc.vector.memset(dst_quad[32:33, :, :], 1.0)
        outsb = singles.tile([128, 4, H * F], FP)

        # ---- stage B: h = x @ W (node-major), esdT = (x @ WA)^T ----
        with tc.tile_pool(name="bigin", bufs=1) as bigin, \
             tc.tile_pool(name="hpsum", bufs=2, space="PSUM") as hpsum:
            xT_sb = bigin.tile([128, KC, N], FP)
            nc.sync.dma_start(xT_sb[:], ap["xT"].rearrange("(k p) n -> p k n", p=128))
            w_sb = bigin.tile([128, KC, H * F], FP)
            nc.sync.dma_start(w_sb[:], ap["w"].rearrange("(k p) f -> p k f", p=128))
            wa_sb = bigin.tile([128, KC, 2 * H], FP)
            nc.sync.dma_start(wa_sb[:], ap["wa"].rearrange("(k p) f -> p k f", p=128))

            for m in range(JT):
                ph = hpsum.tile([128, H * F], FP, tag="ph")
                for k in range(KC):
                    nc.tensor.matmul(
                        ph[:],
                        lhsT=xT_sb[:, k, m * 128:(m + 1) * 128],
                        rhs=w_sb[:, k, :],
                        start=(k == 0),
                        stop=(k == KC - 1),
                    )
                nc.vector.tensor_copy(
                    out=haug_sb[:, m, :, 0:F],
                    in_=ph[:].rearrange("p (h f) -> p h f", h=H),
                )
            for q in range(8):
                pe = hpsum.tile([16, R], FP, tag="pe")
                for k in range(KC):
                    nc.tensor.matmul(
                        pe[:],
                        lhsT=wa_sb[:, k, :],
                        rhs=xT_sb[:, k, q * R:(q + 1) * R],
                        start=(k == 0),
                        stop=(k == KC - 1),
                    )
                nc.vector.tensor_copy(out=esd_sb[:, q * R:(q + 1) * R], in_=pe[:])

        # stage all e_src rows (partitions 0..7) into row 0 of zsrc2 in one
        # SBUF->SBUF DMA.  Compute engines can only address partition bases
        # {0,32,64}; DMA has no such restriction.
        nc.gpsimd.dma_start(
            out=zsrc2[0:1, :, :],
            in_=esd_sb[0:8, 0:R],
        )
        nc.gpsimd.dma_start(
            out=zsrc2[32:33, :, :],
            in_=esd_sb[0:8, 0:R],
        )

        # ---- stage C: masked softmax + alpha @ h, four heads per pass ----
        zpool = ctx.enter_context(tc.tile_pool(name="zpool", bufs=1, space="PSUM"))
        opool = ctx.enter_context(tc.tile_pool(name="opool", bufs=1, space="PSUM"))
        lpool = ctx.enter_context(tc.tile_pool(name="lpool", bufs=2))
        ppool = ctx.enter_context(tc.tile_pool(name="ppool", bufs=2))
        npool = ctx.enter_context(tc.tile_pool(name="npool", bufs=2))

        G = 4
        for _rep in range(repeat):
            for hg in range(H // G):
                h0 = G * hg
                pout = opool.tile([F + 1, G * R], FP, tag="pout")
                for jt in range(JT):
                    pz = zpool.tile([128, G * R], FP, tag="pz")
                    sa = (2 * jt) % 4
                    sb = sa + 1
                    nc.gpsimd.dma_start(
                        out=dst_quad[1:34:32, sa, :],
                        in_=esd_sb[8 + h0:10 + h0, jt * 128:(jt + 1) * 128],
                    )
                    nc.gpsimd.dma_start(
                        out=dst_quad[1:34:32, sb, :],
                        in_=esd_sb[10 + h0:12 + h0, jt * 128:(jt + 1) * 128],
                    )
                    for hl in range(G):
                        b = 32 * (hl % 2)
                        s = sa if hl < 2 else sb
                        nc.tensor.matmul(
                            pz[:, hl * R:(hl + 1) * R],
                            lhsT=dst_quad[b:b + 2, s, :],
                            rhs=zsrc2[b:b + 2, h0 + hl, :],
                            start=True, stop=False, skip_group_check=True,
                        )
                    for hl in range(G):
                        nc.tensor.matmul(
                            pz[:, hl * R:(hl + 1) * R],
                            lhsT=identb_sb[:],
                            rhs=mask_sb[:, jt, :],
                            start=False, stop=True, skip_group_check=True,
                        )
                    zl = lpool.tile([128, G * R], FP, tag="zl")
                    nc.scalar.activation(out=zl[:], in_=pz[:], func=Act.Prelu, alpha=0.2)
                    pp = ppool.tile([128, G * R], FP, tag="pp")
                    nc.scalar.activation(out=pp[:], in_=zl[:], func=Act.Exp)
                    for hl in range(G):
                        sl = slice(hl * R, (hl + 1) * R)
                        nc.tensor.matmul(
                            pout[:, sl],
                            lhsT=haug_sb[:, jt, h0 + hl, :],
                            rhs=pp[:, sl],
                            start=(jt == 0), stop=(jt == JT - 1),
                            skip_group_check=True,
                        )
                # normalize: transpose chunks into bank-aligned slots of the
                # aliased pz scratch (matmul PSUM targets must be
                # bank-aligned), then batched reciprocal + scalar multiply.
                osb = npool.tile([F + 1, G * R], FP, tag="osb")
                nc.vector.tensor_copy(out=osb[:], in_=pout[:])
                for rnd in range(4):
                    pt = zpool.tile([128, G * R], FP, tag="pz")
                    for qq in range(4):
                        q = rnd * 4 + qq
                        nc.tensor.transpose(
                            pt[:, qq * R:qq * R + F + 1],
                            osb[:, q * 128:(q + 1) * 128],
                            identf_sb[0:F + 1, 0:F + 1],
                        )
                    ptv = pt[:].rearrange("p (q c) -> p q c", c=R)
                    rc = npool.tile([128, 4], FP, tag="rc")
                    nc.vector.reciprocal(rc[:], ptv[:, :, F])
                    for qq in range(4):
                        q = rnd * 4 + qq
                        hl, ic = q // 4, q % 4
                        nc.vector.tensor_scalar_mul(
                            outsb[:, ic, (h0 + hl) * F:(h0 + hl + 1) * F],
                            ptv[:, qq, 0:F],
                            rc[:, qq:qq + 1],
                        )

        nc.sync.dma_start(
            out_ap.rearrange("(ic p) f -> p ic f", p=128),
            outsb[:],
        )


def _host_prep(x, edge_index, W, a):
    x = np.asarray(x, np.float32)
    W = np.asarray(W, np.float32)
    a = np.asarray(a, np.float32)
    src = np.asarray(edge_index[0]).astype(np.int64)
    dst = np.asarray(edge_index[1]).astype(np.int64)

    A = np.zeros((H * F, 2 * H), np.float32)
    for h in range(H):
        A[h * F:(h + 1) * F, h] = a[h, :F]
        A[h * F:(h + 1) * F, 8 + h] = a[h, F:]
    wa = np.ascontiguousarray(W @ A)

    maskadd = np.full((NCORES, N, R), NEG, np.float32)
    c_of = src // R
    i_loc = src % R
    r = (dst - c_of * R) % N
    maskadd[c_of, r, i_loc] = 0.0
    idx = np.arange(R)
    maskadd[:, idx, idx] = 0.0
    maskadd = maskadd.astype(ml_dtypes.bfloat16)

    identb = np.eye(128, dtype=ml_dtypes.bfloat16)
    identf = np.eye(128, dtype=np.float32)
    onesh = np.ones((1, H, R), np.float32)

    in_maps = []
    for c in range(NCORES):
        xT_c = np.ascontiguousarray(np.roll(x, -c * R, axis=0).T)
        in_maps.append({
            "xT": xT_c,
            "w": W,
            "wa": wa,
            "maskadd": np.ascontiguousarray(maskadd[c]),
            "identb": identb,
            "identf": identf,
            "onesh": onesh,
        })
    return in_maps


_CACHED = {}


def _get_program(repeat=1):
    if repeat not in _CACHED:
        _CACHED[repeat] = _build_program(repeat)
    return _CACHED[repeat]


def kernel(x, edge_index, W, a, _repeat=1):
    nc = _get_program(_repeat)
    in_maps = _host_prep(x, edge_index, W, a)
    res = run_bass_kernel_spmd(nc, in_maps, core_ids=list(range(NCORES)))
    out = np.concatenate([res.results[c]["out"] for c in range(NCORES)], axis=0)
    return out.astype(np.float32)



# revision 4
# speedup vs baseline: 3.2949x; 3.2949x over previous
"""Trainium2 Bass kernel for CustomGATConv (dense masked GAT attention).

Strategy (8-core SPMD, row-sharded attention; v2 — instruction-count
minimal for this runtime, where every compute-engine instruction costs
~50-130us regardless of size while DMAs cost ~3us):

  - Each core owns 512 destination rows i (natural order, no rotation).
  - e_src/e_dst are computed exactly on the host (tiny GEMM x @ (W A)).
  - h = x @ W runs on PE in bf16 once per call (stage B).
  - Per rep (stage C), the [j=4096, h=8, i=512] logit tensor is built in
    SBUF by two bulk DVE adds using stride-0 broadcast access patterns
    (e_src broadcast over j-tiles, e_dst broadcast over i, additive
    -200 mask broadcast over heads) — no PSUM, no identity matmuls.
    Prelu and Exp run as two big ACT instructions per 2-j-tile group.
  - alpha @ h (with a ones-column for the softmax denominator) is 256
    bf16 matmuls accumulating into a single [65, 8, 512] PSUM tile that
    occupies all 8 banks across the whole j sweep.
  - Normalization: reciprocal of the denominator row, partition-doubling
    DMA broadcast, one DVE multiply, and a single strided store DMA.
"""

import re

import numpy as np
import ml_dtypes

import bass_rust as br
import concourse.bass as bass
import concourse.tile as tile
from concourse import mybir
from concourse.bass_utils import run_bass_kernel_spmd

N = 4096
IN = 256
H = 8
F = 64
NCORES = 8
R = N // NCORES          # 512 destination rows per core
JT = N // 128            # 32 j-tiles
JTG = 2                  # j-tiles per stage-C group
NEG = -200.0             # additive mask value
FP = mybir.dt.float32
BF = mybir.dt.bfloat16

NORM_MODE = "dma"        # "dma": strided store; "pe": PE-transpose fallback


class _TileContext(tile.TileContext):
    """TileContext whose final drain splits its semaphore waits one per
    instruction — this walrus's CTRL_NO encoding only fits one sync wait."""

    def _drain_and_barrier(self, tick_clock, wait_clock):
        gc = tick_clock.global_clock
        vals = list(map(int, re.findall(r"\d+", repr(gc))))
        nonzero = [(i, t) for i, t in enumerate(vals) if t > 0]
        prev = br.VectorClock()
        partial = br.VectorClock()
        for i, t in nonzero:
            partial.require_at_least(i, t)
            inst = self.nc.sync.drain().ins
            wait_clock.add_sem_waits(
                inst,
                br.ScopedClock({None: partial.copy()}),
                br.ScopedClock({None: prev.copy()}),
            )
            prev = partial.copy()
        drain_inst = self.nc.sync.drain().ins
        wait_clock.add_sem_waits(
            drain_inst,
            br.ScopedClock({None: gc}),
            br.ScopedClock({None: prev.copy()}),
        )
        self.nc.all_engine_barrier()
        popped = self.nc._tile_sem_poison_stack.pop()
        assert popped is self._sem_poison
        self.nc.clear_and_free_semaphores(list(self.sems.allocated().values()))
        self.nc.all_engine_barrier()


def _split_excess_waits(nc, cap_compute=1, cap_nop=1):
    """This walrus encodes at most ~2 sync waits per compute instruction and
    1 per CTRL_NO (nop/drain).  Move excess waits onto injected same-engine
    nops placed immediately before the over-subscribed instruction."""
    n_split = 0
    for fn in nc.m.functions:
        for bb in fn.blocks:
            lst = bb.instructions
            i = 0
            while i < len(lst):
                inst = lst[i]
                si = inst.sync_info
                waits = list(si.on_wait) if si is not None else []
                is_ctrl = isinstance(inst, (mybir.InstNoOp, mybir.InstDrain))
                cap = cap_nop if is_ctrl else cap_compute
                if len(waits) > cap:
                    excess, keep = waits[:-cap], waits[-cap:]
                    for w in excess:
                        nop = mybir.InstNoOp(name=f"waitsplit-{nc.next_id()}")
                        nop.engine = inst.engine
                        nop.sync_info = br.SyncInfo(on_wait=[w], on_update=[])
                        lst.insert(i, nop)
                        i += 1
                        n_split += 1
                    inst.sync_info = br.SyncInfo(
                        on_wait=keep, on_update=list(si.on_update)
                    )
                i += 1
    return n_split


def _build_program(repeat=1):
    nc = bass.Bass("TRN2", target_bir_lowering=False, debug=False)
    ap = {}
    ap["xT"] = nc.dram_tensor("xT", [IN, N], BF, kind="ExternalInput").ap()
    ap["w"] = nc.dram_tensor("w", [IN, H * F], BF, kind="ExternalInput").ap()
    ap["maskadd"] = nc.dram_tensor("maskadd", [N, R], BF, kind="ExternalInput").ap()
    ap["esrcb"] = nc.dram_tensor("esrcb", [128, H, R], BF, kind="ExternalInput").ap()
    ap["edst"] = nc.dram_tensor("edst", [N, H], BF, kind="ExternalInput").ap()
    if NORM_MODE == "pe":
        ap["identf"] = nc.dram_tensor("identf", [128, 128], FP, kind="ExternalInput").ap()
    out_ap = nc.dram_tensor("out", [F, H * R], FP, kind="ExternalOutput").ap()

    with _TileContext(nc) as tc:
        _emit(tc, nc, ap, out_ap, repeat)
    _split_excess_waits(nc)
    return nc


def _emit(tc, nc, ap, out_ap, repeat):
    from contextlib import ExitStack

    Act = mybir.ActivationFunctionType
    with ExitStack() as ctx:
        singles = ctx.enter_context(tc.tile_pool(name="singles", bufs=1))

        # ---- persistent tiles ----
        mask_sb = singles.tile([128, JT, 1, R], BF)
        nc.sync.dma_start(
            mask_sb[:, :, 0, :], ap["maskadd"].rearrange("(t p) i -> p t i", p=128)
        )
        esrcb_sb = singles.tile([128, 1, H, R], BF)
        nc.sync.dma_start(esrcb_sb[:, 0, :, :], ap["esrcb"])
        edst_sb = singles.tile([128, JT, H, 1], BF)
        nc.sync.dma_start(
            edst_sb[:, :, :, 0], ap["edst"].rearrange("(t p) h -> p t h", p=128)
        )
        haug_sb = singles.tile([128, JT, H, F + 1], BF)
        nc.vector.memset(haug_sb[:, :, :, F:F + 1], 1.0)
        if NORM_MODE == "pe":
            identf_sb = singles.tile([128, 128], FP)
            nc.sync.dma_start(identf_sb[:], ap["identf"])

        # ---- stage B: h = x @ W (bf16), packed into haug ----
        with tc.tile_pool(name="bigin", bufs=1) as bigin, \
             tc.tile_pool(name="hpsum", bufs=2, space="PSUM") as hpsum:
            xT_sb = bigin.tile([128, 2, N], BF)
            nc.sync.dma_start(xT_sb[:], ap["xT"].rearrange("(k p) n -> p k n", p=128))
            w_sb = bigin.tile([128, 2, H * F], BF)
            nc.sync.dma_start(w_sb[:], ap["w"].rearrange("(k p) f -> p k f", p=128))

            for mb in range(JT // 4):
                ph = hpsum.tile([128, 4, H * F], FP, tag="ph")
                for mq in range(4):
                    m = mb * 4 + mq
                    for k in range(2):
                        nc.tensor.matmul(
                            ph[:, mq, :],
                            lhsT=xT_sb[:, k, m * 128:(m + 1) * 128],
                            rhs=w_sb[:, k, :],
                            start=(k == 0),
                            stop=(k == 1),
                        )
                nc.vector.tensor_copy(
                    out=haug_sb[:, mb * 4:(mb + 1) * 4, :, 0:F],
                    in_=ph[:].rearrange("p q (h f) -> p q h f", h=H),
                )

        # ---- stage C: masked softmax + alpha @ h ----
        opool = ctx.enter_context(tc.tile_pool(name="opool", bufs=1, space="PSUM"))
        zpool = ctx.enter_context(tc.tile_pool(name="zpool", bufs=2))
        ppool = ctx.enter_context(tc.tile_pool(name="ppool", bufs=2))
        npool = ctx.enter_context(tc.tile_pool(name="npool", bufs=2))

        bshape = [128, JTG, H, R]
        for _rep in range(repeat):
            pout = opool.tile([F + 1, H, R], FP, tag="pout")
            for g in range(JT // JTG):
                t0 = g * JTG
                zt = zpool.tile(bshape, BF, tag="zt")
                nc.vector.tensor_tensor(
                    out=zt[:],
                    in0=esrcb_sb[:].broadcast_to(bshape),
                    in1=edst_sb[:, t0:t0 + JTG, :, :].broadcast_to(bshape),
                    op=mybir.AluOpType.add,
                )
                zp = ppool.tile(bshape, BF, tag="zp")
                nc.vector.tensor_tensor(
                    out=zp[:],
                    in0=zt[:],
                    in1=mask_sb[:, t0:t0 + JTG, :, :].broadcast_to(bshape),
                    op=mybir.AluOpType.add,
                )
                zpf = zp[:].rearrange("p t h i -> p (t h i)")
                ztf = zt[:].rearrange("p t h i -> p (t h i)")
                nc.scalar.activation(out=ztf, in_=zpf, func=Act.Prelu, alpha=0.2)
                nc.scalar.activation(out=zpf, in_=ztf, func=Act.Exp)
                for tl in range(JTG):
                    jt = t0 + tl
                    for h in range(H):
                        nc.tensor.matmul(
                            pout[:, h, :],
                            lhsT=haug_sb[:, jt, h, :],
                            rhs=zp[:, tl, h, :],
                            start=(jt == 0),
                            stop=(jt == JT - 1),
                            skip_group_check=True,
                        )

            # ---- normalize + store ----
            rcp = npool.tile([64, H, R], FP, tag="rcp")
            nc.vector.reciprocal(rcp[0:1, :, :], pout[F:F + 1, :, :])
            for d in (1, 2, 4, 8, 16, 32):
                nc.gpsimd.dma_start(out=rcp[d:2 * d], in_=rcp[0:d])
            osb = npool.tile([64, H, R], FP, tag="osb")
            nc.vector.tensor_tensor(
                out=osb[:],
                in0=pout[0:F, :, :],
                in1=rcp[:],
                op=mybir.AluOpType.mult,
            )
            nc.sync.dma_start(
                out_ap, osb[:].rearrange("f h i -> f (h i)")
            )


def _host_prep(x, edge_index, W, a):
    x = np.asarray(x, np.float32)
    W = np.asarray(W, np.float32)
    a = np.asarray(a, np.float32)
    src = np.asarray(edge_index[0]).astype(np.int64)
    dst = np.asarray(edge_index[1]).astype(np.int64)

    # exact e_src / e_dst on host: e = x @ (W A)
    A = np.zeros((H * F, 2 * H), np.float32)
    for h in range(H):
        A[h * F:(h + 1) * F, h] = a[h, :F]
        A[h * F:(h + 1) * F, 8 + h] = a[h, F:]
    ea = x @ (W @ A)                       # [N, 16]
    esrc = ea[:, :H]                       # [N, 8]
    edst = np.ascontiguousarray(ea[:, H:]).astype(ml_dtypes.bfloat16)  # [N, 8]

    # additive mask, full matrix [j, i]: 0 iff edge (src=i, dst=j) or i==j
    mfull = np.full((N, N), NEG, ml_dtypes.bfloat16)
    mfull[dst, src] = 0.0
    idx = np.arange(N)
    mfull[idx, idx] = 0.0

    xTb = np.ascontiguousarray(x.T.astype(ml_dtypes.bfloat16))
    wb = W.astype(ml_dtypes.bfloat16)

    in_maps = []
    for c in range(NCORES):
        sl = slice(c * R, (c + 1) * R)
        esrcb = np.ascontiguousarray(
            np.broadcast_to(
                esrc[sl].T.astype(ml_dtypes.bfloat16)[None], (128, H, R)
            )
        )
        m = {
            "xT": xTb,
            "w": wb,
            "maskadd": np.ascontiguousarray(mfull[:, sl]),
            "esrcb": esrcb,
            "edst": edst,
        }
        if NORM_MODE == "pe":
            m["identf"] = np.eye(128, dtype=np.float32)
        in_maps.append(m)
    return in_maps


_CACHED = {}


def _get_program(repeat=1):
    if repeat not in _CACHED:
        _CACHED[repeat] = _build_program(repeat)
    return _CACHED[repeat]


def kernel(x, edge_index, W, a, _repeat=1):
    nc = _get_program(_repeat)
    in_maps = _host_prep(x, edge_index, W, a)
    res = run_bass_kernel_spmd(nc, in_maps, core_ids=list(range(NCORES)))
    # device output is [64 f, 8 h, 512 i] per core; transpose to [i, (h f)]
    out = np.empty((N, H * F), np.float32)
    for c in range(NCORES):
        arr = res.results[c]["out"].reshape(F, H, R)
        out[c * R:(c + 1) * R] = (
            np.transpose(arr, (2, 1, 0)).reshape(R, H * F)
        )
    return out
